# revision 1
# baseline (speedup 1.0000x reference)
"""Trainium2 Bass kernel for the MLPConstructor2 adjacency problem.

Computes, per batch b (one NeuronCore each, 8-way data parallel over B):
    adj[i, j] = tanh(relu(x1_i @ w1 + x2_j @ w2 + b))
for the four (spatial/temporal) quadrants of a (2560, 2560) output.

The output is an outer broadcast-sum of per-row and per-column scalar
vectors, so the kernel is HBM-write bound (26.2 MB/core). Design:

- x is staged twice, in (t p) layout for the row-side stats (so each
  128-row output tile's biases land on partitions directly) and in (p t)
  layout for the col-side stats (so the stat tile streams out to a DRAM
  scratch contiguously in row order -- no transpose anywhere).
- All eight dot-product vectors are mul(+step-0-broadcast weight)/reduce
  on VectorE; quadrant biases are folded into the col-side stats.
- The column vectors are replicated across partitions with a single
  partition-step-0 DMA broadcast-load of the scratch: pure DMA broadcast.
  No PE, no PSUM (fp32 PE matmuls are 4 cycles/row and cold-clocked).
- Main loop per 128-row output tile: 2 ScalarE tanh activations
  (per-quadrant per-partition row bias), 1 VectorE relu in place, one
  1.31 MB contiguous store, alternating Sync (HWDGE) / GpSimd (SWDGE)
  rings so two DMA queues drain in parallel.
"""

import numpy as np
from contextlib import ExitStack

import concourse.bacc as bacc
import concourse.mybir as mybir
import concourse.tile as tile
from concourse.bass_utils import run_bass_kernel_spmd

B, N, T, D = 8, 2048, 512, 32
W = N + T            # 2560
NT, TT = N // 128, T // 128   # 16, 4 row-tiles
F32 = mybir.dt.float32
QUADS = ("ss", "st", "ts", "tt")


def _emit(tc, sp, tm, ws, scr, adj):
    nc = tc.nc
    AF = mybir.ActivationFunctionType
    OP = mybir.AluOpType
    with ExitStack() as ctx:
        const = ctx.enter_context(tc.tile_pool(name="const", bufs=1))
        outp = ctx.enter_context(tc.tile_pool(name="outp", bufs=8))

        # ---- stage inputs, col-stat layout first (critical path) ----------
        # (p t): row p*nt+t at [p, t*D:(t+1)*D] -- contiguous 2KB per partition
        x_sp_pt = const.tile([128, NT * D], F32)
        nc.sync.dma_start(x_sp_pt[:], sp.rearrange("(p t) d -> p t d", p=128))
        x_tm_pt = const.tile([128, TT * D], F32)
        nc.sync.dma_start(x_tm_pt[:], tm.rearrange("(p t) d -> p t d", p=128))
        # (t p): row t*128+p at [p, t*D:(t+1)*D] -- for row-side bias tiles
        x_sp_tp = const.tile([128, NT * D], F32)
        nc.sync.dma_start(x_sp_tp[:], sp.rearrange("(t p) d -> p t d", p=128))
        x_tm_tp = const.tile([128, TT * D], F32)
        nc.sync.dma_start(x_tm_tp[:], tm.rearrange("(t p) d -> p t d", p=128))

        # broadcast weights straight from DRAM with step-0 partition APs.
        # col-side pairs: wc_sp = [w_ss2, w_ts2], wc_tm = [w_st2, w_tt2]
        # row-side pairs: wr_sp = [w_ss1, w_st1], wr_tm = [w_ts1, w_tt1]
        def wload(name, spec):
            t = const.tile([128, 2 * D], F32, name=name, tag=name)
            for i, (nm, half) in enumerate(spec):
                src = ws[f"w_{nm}"][half * D : (half + 1) * D]
                nc.scalar.dma_start(
                    t[:, i * D : (i + 1) * D], src.unsqueeze(0).broadcast_to((128, D))
                )
            return t

        bb = const.tile([128, 4], F32)   # b_ss, b_st, b_ts, b_tt broadcast
        for j, nm in enumerate(QUADS):
            nc.scalar.dma_start(
                bb[:, j : j + 1], ws[f"b_{nm}"].unsqueeze(0).broadcast_to((128, 1))
            )
        wc_sp = wload("wc_sp", [("ss", 1), ("ts", 1)])
        wc_tm = wload("wc_tm", [("st", 1), ("tt", 1)])
        wr_sp = wload("wr_sp", [("ss", 0), ("st", 0)])
        wr_tm = wload("wr_tm", [("ts", 0), ("tt", 0)])

        # ---- stats on VectorE: mul + reduce over D ------------------------
        def stats(x, wpair, nt, name, nslots=2, bias=None, store=None):
            # per-slot mul/reduce so downstream stores fire ASAP
            st = const.tile(
                [128, nslots * nt], F32, name=f"stat_{name}", tag=f"stat_{name}"
            )
            prod = const.tile(
                [128, nt * D], F32, name=f"prod_{name}", tag=f"prod_{name}"
            )
            x3 = x[:].rearrange("p (t d) -> p t d", t=nt)
            p3 = prod[:].rearrange("p (t d) -> p t d", t=nt)
            for s in range(nslots):
                w3 = wpair[:, s * D : (s + 1) * D].unsqueeze(1).broadcast_to(
                    (128, nt, D)
                )
                nc.vector.tensor_tensor(p3, x3, w3, OP.mult)
                sl = st[:, s * nt : (s + 1) * nt]
                nc.vector.tensor_reduce(
                    sl, p3, axis=mybir.AxisListType.X, op=OP.add
                )
                if bias is not None:
                    nc.vector.tensor_scalar_add(sl, sl, bias[s])
                if store is not None:
                    nc.sync.dma_start(store[s], sl)
            return st

        # col stats: (p t) layout; emit mul/reduce/bias per slot, store the
        # slot to its scratch range, and immediately queue the broadcast-load
        # for that range so col_sp[0:N] (the first ACT dependency) is never
        # stuck behind unrelated scratch stores in the ring FIFO.
        col_sp = const.tile([128, W], F32)
        col_tm = const.tile([128, W], F32)

        def cstat_slot(x, nt, w, b, scr_rng, col_dst, name):
            prod = const.tile([128, nt * D], F32, name=f"prod_{name}", tag="cprod")
            x3 = x[:].rearrange("p (t d) -> p t d", t=nt)
            p3 = prod[:].rearrange("p (t d) -> p t d", t=nt)
            w3 = w.unsqueeze(1).broadcast_to((128, nt, D))
            nc.vector.tensor_tensor(p3, x3, w3, OP.mult)
            st = const.tile([128, nt], F32, name=f"cstat_{name}", tag=f"cstat_{name}")
            nc.vector.tensor_reduce(st[:], p3, axis=mybir.AxisListType.X, op=OP.add)
            nc.vector.tensor_scalar_add(st[:], st[:], b)
            nc.sync.dma_start(scr_rng, st[:])
            nc.sync.dma_start(
                col_dst, scr_rng.unsqueeze(0).broadcast_to((128, scr_rng.shape[0]))
            )

        cstat_slot(x_sp_pt, NT, wc_sp[:, 0:D], bb[:, 0:1],
                   scr["sp"][0:N], col_sp[:, 0:N], "ss")
        cstat_slot(x_tm_pt, TT, wc_tm[:, 0:D], bb[:, 1:2],
                   scr["sp"][N:W], col_sp[:, N:W], "st")
        cstat_slot(x_sp_pt, NT, wc_sp[:, D : 2 * D], bb[:, 2:3],
                   scr["tm"][0:N], col_tm[:, 0:N], "ts")
        cstat_slot(x_tm_pt, TT, wc_tm[:, D : 2 * D], bb[:, 3:4],
                   scr["tm"][N:W], col_tm[:, N:W], "tt")

        # row stats: (t p) layout, slots [a_ss, a_st] / [a_ts, a_tt]
        r_sp = stats(x_sp_tp, wr_sp, NT, "r_sp")
        r_tm = stats(x_tm_tp, wr_tm, TT, "r_tm")

        # ---- main loop: 20 output row-tiles of [128, 2560] ----------------
        def row_block(k, row0, col, st, nt, t):
            ot = outp.tile([128, W], F32, name=f"ot{k}", tag="ot")
            nc.scalar.activation(
                ot[:, 0:N], col[:, 0:N], AF.Tanh, bias=st[:, t : t + 1]
            )
            nc.scalar.activation(
                ot[:, N:W], col[:, N:W], AF.Tanh, bias=st[:, nt + t : nt + t + 1]
            )
            nc.vector.tensor_scalar_max(ot[:], ot[:], 0.0)
            nc.sync.dma_start(adj[row0 : row0 + 128, :], ot[:])

        for t in range(NT):
            row_block(t, 128 * t, col_sp, r_sp, NT, t)
        for t in range(TT):
            row_block(NT + t, N + 128 * t, col_tm, r_tm, TT, t)


def build_nc(num_devices=8):
    nc = bacc.Bacc(
        "TRN2",
        target_bir_lowering=False,
        debug=False,
        enable_asserts=True,
        num_devices=num_devices,
    )
    sp = nc.dram_tensor("spatial_nodes", (N, D), F32, kind="ExternalInput").ap()
    tm = nc.dram_tensor("temporal_nodes", (T, D), F32, kind="ExternalInput").ap()
    ws = {}
    for nm in QUADS:
        ws[f"w_{nm}"] = nc.dram_tensor(f"w_{nm}", (2 * D,), F32, kind="ExternalInput").ap()
        ws[f"b_{nm}"] = nc.dram_tensor(f"b_{nm}", (1,), F32, kind="ExternalInput").ap()
    scr = {
        "sp": nc.dram_tensor("scr_sp", (W,), F32, kind="Internal").ap(),
        "tm": nc.dram_tensor("scr_tm", (W,), F32, kind="Internal").ap(),
    }
    adj = nc.dram_tensor("adj", (W, W), F32, kind="ExternalOutput").ap()

    with tile.TileContext(nc) as tc:
        _emit(tc, sp, tm, ws, scr, adj)
    nc.compile()
    return nc


def make_in_maps(inputs):
    in_maps = []
    for b in range(B):
        m = {
            "spatial_nodes": np.ascontiguousarray(inputs["spatial_nodes"][b], np.float32),
            "temporal_nodes": np.ascontiguousarray(inputs["temporal_nodes"][b], np.float32),
        }
        for nm in QUADS:
            m[f"w_{nm}"] = np.ascontiguousarray(inputs[f"w_{nm}"], np.float32)
            m[f"b_{nm}"] = np.ascontiguousarray(inputs[f"b_{nm}"], np.float32)
        in_maps.append(m)
    return in_maps


_NC = {}


def run(inputs, trace=False, trace_cores=None):
    if 8 not in _NC:
        _NC[8] = build_nc(8)
    res = run_bass_kernel_spmd(
        _NC[8], make_in_maps(inputs), core_ids=list(range(B)), trace=trace,
        trace_cores=trace_cores,
    )
    out = np.stack([res.results[i]["adj"] for i in range(B)], axis=0)
    return out, res


def kernel(**inputs) -> np.ndarray:
    out, _ = run(inputs, trace=False)
    return out



# revision 5
# speedup vs baseline: 1.0302x; 1.0302x over previous
"""Trainium2 Bass kernel for the MLPConstructor2 adjacency problem.

Computes, per batch b (one NeuronCore each, 8-way data parallel over B):
    adj[i, j] = tanh(relu(x1_i @ w1 + x2_j @ w2 + b))
for the four (spatial/temporal) quadrants of a (2560, 2560) output.

v2 design (ACT-bound, ~46us ScalarE floor):
- Output is stored as fp16 (tolerance is 2e-2; fp16 rounding adds ~5e-4),
  halving the HBM store traffic to 13.1 MB/core.
- Per 256-row block: VectorE computes max(col + row_scalar, 0) with ONE
  fused tensor_scalar (add, max) per quadrant range into an fp32 tmp tile;
  ScalarE then runs a single TANH over the whole [128, 5120] block
  (fp32 in -> fp16 out), and one 1.31 MB store writes 256 rows
  (partition p holds rows 2p, 2p+1 -> 10 KB contiguous per partition).
- tanh(relu(x)) == relu(tanh(x)); we apply relu first so the tanh is the
  last op and its fp16 output stores directly.
- Col-side stats per quadrant pair are computed in (p t) layout, stored
  once to a DRAM scratch in [128, 20]-interleaved order, and broadcast
  back to all partitions with step-0 partition APs (pure DMA broadcast).
- A dummy 1-elem tanh at t=0 pulls the ACT_TABLE_LOAD off the critical
  path; weight/bias broadcasts ride the otherwise-idle GpSimd (SWDGE)
  queue so ScalarE does nothing but TANH.
"""

import numpy as np
from contextlib import ExitStack

import concourse.bacc as bacc
import concourse.mybir as mybir
import concourse.tile as tile
from concourse.bass_utils import run_bass_kernel_spmd

B, N, T, D = 8, 2048, 512, 32
W = N + T                     # 2560
NT, TT = N // 128, T // 128   # 16, 4 col-stat slots
GN, GT = N // 256, T // 256   # 8, 2 output row blocks (256 rows each)
F32 = mybir.dt.float32
F16 = mybir.dt.float16
QUADS = ("ss", "st", "ts", "tt")


def _emit(tc, sp, tm, ws, scr, adj):
    nc = tc.nc
    AF = mybir.ActivationFunctionType
    OP = mybir.AluOpType
    with ExitStack() as ctx:
        const = ctx.enter_context(tc.tile_pool(name="const", bufs=1))
        tmpp = ctx.enter_context(tc.tile_pool(name="tmpp", bufs=2))
        outp = ctx.enter_context(tc.tile_pool(name="outp", bufs=3))

        # ---- ACT table-load warmup: first tanh pays ~2.7us once ----------
        warm = const.tile([128, 1], F32)
        nc.vector.memset(warm[:], 0.0)
        nc.scalar.activation(warm[:], warm[:], AF.Tanh)

        # ---- stage inputs -------------------------------------------------
        # (p t): row p*nt+t at [p, t*D:(t+1)*D] -- for col-side stats
        x_sp_pt = const.tile([128, NT * D], F32)
        nc.sync.dma_start(x_sp_pt[:], sp.rearrange("(p t) d -> p t d", p=128))
        x_tm_pt = const.tile([128, TT * D], F32)
        nc.sync.dma_start(x_tm_pt[:], tm.rearrange("(p t) d -> p t d", p=128))
        # (g p k): row g*256 + 2p + k at [p, (g*2+k)*D...] -- row-side stats
        # laid out so block g's two row scalars are slots 2g, 2g+1.
        x_sp_r = const.tile([128, NT * D], F32)
        nc.sync.dma_start(
            x_sp_r[:].rearrange("p (g k d) -> p g k d", g=GN, k=2),
            sp.rearrange("(g p k) d -> p g k d", g=GN, p=128),
        )
        x_tm_r = const.tile([128, TT * D], F32)
        nc.sync.dma_start(
            x_tm_r[:].rearrange("p (g k d) -> p g k d", g=GT, k=2),
            tm.rearrange("(g p k) d -> p g k d", g=GT, p=128),
        )

        # broadcast weights straight from DRAM with step-0 partition APs,
        # on the GpSimd (SWDGE) queue so Sync/Scalar stay free.
        # col-side pairs: wc_sp = [w_ss2, w_ts2], wc_tm = [w_st2, w_tt2]
        # row-side pairs: wr_sp = [w_ss1, w_st1], wr_tm = [w_ts1, w_tt1]
        def wload(name, spec):
            t = const.tile([128, 2 * D], F32, name=name, tag=name)
            for i, (nm, half) in enumerate(spec):
                src = ws[f"w_{nm}"][half * D : (half + 1) * D]
                nc.gpsimd.dma_start(
                    t[:, i * D : (i + 1) * D], src.unsqueeze(0).broadcast_to((128, D))
                )
            return t

        wc_sp = wload("wc_sp", [("ss", 1), ("st", 1)])   # cols seen by spatial rows
        wc_tm = wload("wc_tm", [("ts", 1), ("tt", 1)])   # cols seen by temporal rows
        bb = const.tile([128, 4], F32)   # b_ss, b_st, b_ts, b_tt broadcast
        for j, nm in enumerate(QUADS):
            nc.gpsimd.dma_start(
                bb[:, j : j + 1], ws[f"b_{nm}"].unsqueeze(0).broadcast_to((128, 1))
            )
        wr_sp = wload("wr_sp", [("ss", 0), ("st", 0)])
        wr_tm = wload("wr_tm", [("ts", 0), ("tt", 0)])

        # ---- stats on VectorE: mul + reduce over D ------------------------
        def mulred(x, nt, w, dst, name):
            prod = const.tile([128, nt * D], F32, name=f"prod_{name}", tag="prod")
            x3 = x[:].rearrange("p (t d) -> p t d", t=nt)
            p3 = prod[:].rearrange("p (t d) -> p t d", t=nt)
            w3 = w.unsqueeze(1).broadcast_to((128, nt, D))
            nc.vector.tensor_tensor(p3, x3, w3, OP.mult)
            nc.vector.tensor_reduce(dst, p3, axis=mybir.AxisListType.X, op=OP.add)

        # col stats for one output half (ss+st -> col_sp, ts+tt -> col_tm):
        # [128, 20] stat tile (16 spatial slots + 4 temporal, quadrant biases
        # folded in), ONE store to scr in p-major [p*20 + j] order, then two
        # partition-broadcast loads rebuild the [128, W] col vector.
        def col_half(wsp_sl, wtm_sl, b_sp, b_tm, scr_t, name):
            cst = const.tile([128, NT + TT], F32, name=f"cst_{name}", tag=f"cst_{name}")
            mulred(x_sp_pt, NT, wsp_sl, cst[:, 0:NT], f"c{name}s")
            nc.vector.tensor_scalar_add(cst[:, 0:NT], cst[:, 0:NT], b_sp)
            mulred(x_tm_pt, TT, wtm_sl, cst[:, NT:], f"c{name}t")
            nc.vector.tensor_scalar_add(cst[:, NT:], cst[:, NT:], b_tm)
            nc.sync.dma_start(scr_t.rearrange("(p j) -> p j", p=128), cst[:])
            col = const.tile([128, W], F32, name=f"col_{name}", tag=f"col_{name}")
            s3 = scr_t.rearrange("(p j) -> p j", p=128)
            nc.sync.dma_start(
                col[:, 0:N],
                s3[:, 0:NT].unsqueeze(0).broadcast_to((128, 128, NT)),
            )
            nc.sync.dma_start(
                col[:, N:W],
                s3[:, NT:].unsqueeze(0).broadcast_to((128, 128, TT)),
            )
            return col

        col_sp = col_half(wc_sp[:, 0:D], wc_sp[:, D:], bb[:, 0:1], bb[:, 1:2],
                          scr["sp"], "sp")
        col_tm = col_half(wc_tm[:, 0:D], wc_tm[:, D:], bb[:, 2:3], bb[:, 3:4],
                          scr["tm"], "tm")

        # row stats: slots [a_q1 (0:nt), a_q2 (nt:2nt)], block g rows = 2g, 2g+1
        r_sp = const.tile([128, 2 * NT], F32)
        mulred(x_sp_r, NT, wr_sp[:, 0:D], r_sp[:, 0:NT], "r_ss")
        mulred(x_sp_r, NT, wr_sp[:, D:], r_sp[:, NT:], "r_st")
        r_tm = const.tile([128, 2 * TT], F32)
        mulred(x_tm_r, TT, wr_tm[:, 0:D], r_tm[:, 0:TT], "r_ts")
        mulred(x_tm_r, TT, wr_tm[:, D:], r_tm[:, TT:], "r_tt")

        # ---- main loop: 10 output blocks of 256 rows ----------------------
        def block(k, row0, col, rst, nt, g):
            tmp = tmpp.tile([128, 2 * W], F32, name=f"tmp{k}", tag="tmp")
            ot = outp.tile([128, 2 * W], F16, name=f"ot{k}", tag="ot")
            for h in range(2):          # row 2g+h on partition p
                o = h * W
                rn = rst[:, 2 * g + h : 2 * g + h + 1]
                rt = rst[:, nt + 2 * g + h : nt + 2 * g + h + 1]
                nc.vector.tensor_scalar(
                    tmp[:, o : o + N], col[:, 0:N], rn, 0.0, OP.add, OP.max
                )
                nc.vector.tensor_scalar(
                    tmp[:, o + N : o + W], col[:, N:W], rt, 0.0, OP.add, OP.max
                )
            nc.scalar.activation(ot[:], tmp[:], AF.Tanh)
            nc.sync.dma_start(
                adj[row0 : row0 + 256, :].rearrange("(p t) w -> p (t w)", p=128),
                ot[:],
            )

        for g in range(GN):
            block(g, 256 * g, col_sp, r_sp, NT, g)
        for g in range(GT):
            block(GN + g, N + 256 * g, col_tm, r_tm, TT, g)


def build_nc(num_devices=8):
    nc = bacc.Bacc(
        "TRN2",
        target_bir_lowering=False,
        debug=False,
        enable_asserts=True,
        num_devices=num_devices,
    )
    sp = nc.dram_tensor("spatial_nodes", (N, D), F32, kind="ExternalInput").ap()
    tm = nc.dram_tensor("temporal_nodes", (T, D), F32, kind="ExternalInput").ap()
    ws = {}
    for nm in QUADS:
        ws[f"w_{nm}"] = nc.dram_tensor(f"w_{nm}", (2 * D,), F32, kind="ExternalInput").ap()
        ws[f"b_{nm}"] = nc.dram_tensor(f"b_{nm}", (1,), F32, kind="ExternalInput").ap()
    scr = {
        "sp": nc.dram_tensor("scr_sp", (W,), F32, kind="Internal").ap(),
        "tm": nc.dram_tensor("scr_tm", (W,), F32, kind="Internal").ap(),
    }
    adj = nc.dram_tensor("adj", (W, W), F16, kind="ExternalOutput").ap()

    with tile.TileContext(nc) as tc:
        _emit(tc, sp, tm, ws, scr, adj)
    nc.compile()
    return nc


def make_in_maps(inputs):
    in_maps = []
    for b in range(B):
        m = {
            "spatial_nodes": np.ascontiguousarray(inputs["spatial_nodes"][b], np.float32),
            "temporal_nodes": np.ascontiguousarray(inputs["temporal_nodes"][b], np.float32),
        }
        for nm in QUADS:
            m[f"w_{nm}"] = np.ascontiguousarray(inputs[f"w_{nm}"], np.float32)
            m[f"b_{nm}"] = np.ascontiguousarray(inputs[f"b_{nm}"], np.float32)
        in_maps.append(m)
    return in_maps


_NC = {}


def run(inputs, trace=False, trace_cores=None):
    if 8 not in _NC:
        _NC[8] = build_nc(8)
    res = run_bass_kernel_spmd(
        _NC[8], make_in_maps(inputs), core_ids=list(range(B)), trace=trace,
        trace_cores=trace_cores,
    )
    out = np.stack(
        [res.results[i]["adj"].astype(np.float32) for i in range(B)], axis=0
    )
    return out, res


def kernel(**inputs) -> np.ndarray:
    out, _ = run(inputs, trace=False)
    return out


# revision 6
# speedup vs baseline: 1.1068x; 1.0743x over previous
"""Trainium2 Bass kernel for the MLPConstructor2 adjacency problem.

Computes, per batch b (one NeuronCore each, 8-way data parallel over B):
    adj[i, j] = tanh(relu(x1_i @ w1 + x2_j @ w2 + b))
for the four (spatial/temporal) quadrants of a (2560, 2560) output.

v2 design (ACT-bound, ~46us ScalarE floor):
- Output is stored as fp16 (tolerance is 2e-2; fp16 rounding adds ~5e-4),
  halving the HBM store traffic to 13.1 MB/core.
- Per 256-row block: VectorE computes max(col + row_scalar, 0) with ONE
  fused tensor_scalar (add, max) per quadrant range into an fp32 tmp tile;
  ScalarE then runs a single TANH over the whole [128, 5120] block
  (fp32 in -> fp16 out), and one 1.31 MB store writes 256 rows
  (partition p holds rows 2p, 2p+1 -> 10 KB contiguous per partition).
- tanh(relu(x)) == relu(tanh(x)); we apply relu first so the tanh is the
  last op and its fp16 output stores directly.
- Col-side stats per quadrant pair are computed in (p t) layout, stored
  once to a DRAM scratch in [128, 20]-interleaved order, and broadcast
  back to all partitions with step-0 partition APs (pure DMA broadcast).
- A dummy 1-elem tanh at t=0 pulls the ACT_TABLE_LOAD off the critical
  path; weight/bias broadcasts ride the otherwise-idle GpSimd (SWDGE)
  queue so ScalarE does nothing but TANH.
"""

import numpy as np
from contextlib import ExitStack

import concourse.bacc as bacc
import concourse.mybir as mybir
import concourse.tile as tile
from concourse.bass_utils import run_bass_kernel_spmd

B, N, T, D = 8, 2048, 512, 32
W = N + T                     # 2560
NT, TT = N // 128, T // 128   # 16, 4 col-stat slots
GN, GT = N // 256, T // 256   # 8, 2 output row blocks (256 rows each)
F32 = mybir.dt.float32
F16 = mybir.dt.float16
QUADS = ("ss", "st", "ts", "tt")


def _emit(tc, sp, tm, ws, scr, adj):
    nc = tc.nc
    AF = mybir.ActivationFunctionType
    OP = mybir.AluOpType
    with ExitStack() as ctx:
        const = ctx.enter_context(tc.tile_pool(name="const", bufs=1))
        tmpp = ctx.enter_context(tc.tile_pool(name="tmpp", bufs=2))
        outp = ctx.enter_context(tc.tile_pool(name="outp", bufs=3))

        # ---- ACT table-load warmup: first tanh pays ~2.7us once ----------
        warm = const.tile([128, 1], F32)
        nc.vector.memset(warm[:], 0.0)
        nc.scalar.activation(warm[:], warm[:], AF.Tanh)

        # ---- stage inputs -------------------------------------------------
        # (p t): row p*nt+t at [p, t*D:(t+1)*D] -- for col-side stats
        x_sp_pt = const.tile([128, NT * D], F32)
        nc.sync.dma_start(x_sp_pt[:], sp.rearrange("(p t) d -> p t d", p=128))
        x_tm_pt = const.tile([128, TT * D], F32)
        nc.sync.dma_start(x_tm_pt[:], tm.rearrange("(p t) d -> p t d", p=128))
        # (g p k): row g*256 + 2p + k at [p, (g*2+k)*D...] -- row-side stats
        # laid out so block g's two row scalars are slots 2g, 2g+1.
        x_sp_r = const.tile([128, NT * D], F32)
        nc.sync.dma_start(
            x_sp_r[:].rearrange("p (g k d) -> p g k d", g=GN, k=2),
            sp.rearrange("(g p k) d -> p g k d", g=GN, p=128),
        )
        x_tm_r = const.tile([128, TT * D], F32)
        nc.sync.dma_start(
            x_tm_r[:].rearrange("p (g k d) -> p g k d", g=GT, k=2),
            tm.rearrange("(g p k) d -> p g k d", g=GT, p=128),
        )

        # broadcast weights straight from DRAM with step-0 partition APs,
        # on the GpSimd (SWDGE) queue so Sync/Scalar stay free.
        # col-side pairs: wc_sp = [w_ss2, w_ts2], wc_tm = [w_st2, w_tt2]
        # row-side pairs: wr_sp = [w_ss1, w_st1], wr_tm = [w_ts1, w_tt1]
        def wload(name, spec):
            t = const.tile([128, 2 * D], F32, name=name, tag=name)
            for i, (nm, half) in enumerate(spec):
                src = ws[f"w_{nm}"][half * D : (half + 1) * D]
                nc.gpsimd.dma_start(
                    t[:, i * D : (i + 1) * D], src.unsqueeze(0).broadcast_to((128, D))
                )
            return t

        wc_sp = wload("wc_sp", [("ss", 1), ("st", 1)])   # cols seen by spatial rows
        wc_tm = wload("wc_tm", [("ts", 1), ("tt", 1)])   # cols seen by temporal rows
        bb = const.tile([128, 4], F32)   # b_ss, b_st, b_ts, b_tt broadcast
        for j, nm in enumerate(QUADS):
            nc.gpsimd.dma_start(
                bb[:, j : j + 1], ws[f"b_{nm}"].unsqueeze(0).broadcast_to((128, 1))
            )
        wr_sp = wload("wr_sp", [("ss", 0), ("st", 0)])
        wr_tm = wload("wr_tm", [("ts", 0), ("tt", 0)])

        # ---- stats on VectorE: mul + reduce over D ------------------------
        def mulred(x, nt, w, dst, name):
            prod = const.tile([128, nt * D], F32, name=f"prod_{name}", tag="prod")
            x3 = x[:].rearrange("p (t d) -> p t d", t=nt)
            p3 = prod[:].rearrange("p (t d) -> p t d", t=nt)
            w3 = w.unsqueeze(1).broadcast_to((128, nt, D))
            nc.vector.tensor_tensor(p3, x3, w3, OP.mult)
            nc.vector.tensor_reduce(dst, p3, axis=mybir.AxisListType.X, op=OP.add)

        # col stats for one output half (ss+st -> col_sp, ts+tt -> col_tm):
        # [128, 20] stat tile (16 spatial slots + 4 temporal, quadrant biases
        # folded in), stored to scr keeping DRAM contiguous per range so the
        # partition-broadcast reload reads ONE contiguous chunk per partition
        # (128 big descriptors, not 16k tiny ones).
        def col_half(wsp_sl, wtm_sl, b_sp, b_tm, scr_t, name):
            cst = const.tile([128, NT + TT], F32, name=f"cst_{name}", tag=f"cst_{name}")
            col = const.tile([128, W], F32, name=f"col_{name}", tag=f"col_{name}")
            mulred(x_sp_pt, NT, wsp_sl, cst[:, 0:NT], f"c{name}s")
            nc.vector.tensor_scalar_add(cst[:, 0:NT], cst[:, 0:NT], b_sp)
            nc.sync.dma_start(
                scr_t[0:N].rearrange("(p j) -> p j", p=128), cst[:, 0:NT]
            )
            nc.sync.dma_start(
                col[:, 0:N], scr_t[0:N].unsqueeze(0).broadcast_to((128, N))
            )
            mulred(x_tm_pt, TT, wtm_sl, cst[:, NT:], f"c{name}t")
            nc.vector.tensor_scalar_add(cst[:, NT:], cst[:, NT:], b_tm)
            nc.sync.dma_start(
                scr_t[N:W].rearrange("(p j) -> p j", p=128), cst[:, NT:]
            )
            nc.sync.dma_start(
                col[:, N:W], scr_t[N:W].unsqueeze(0).broadcast_to((128, T))
            )
            return col

        col_sp = col_half(wc_sp[:, 0:D], wc_sp[:, D:], bb[:, 0:1], bb[:, 1:2],
                          scr["sp"], "sp")
        col_tm = col_half(wc_tm[:, 0:D], wc_tm[:, D:], bb[:, 2:3], bb[:, 3:4],
                          scr["tm"], "tm")

        # row stats: slots [a_q1 (0:nt), a_q2 (nt:2nt)], block g rows = 2g, 2g+1
        r_sp = const.tile([128, 2 * NT], F32)
        mulred(x_sp_r, NT, wr_sp[:, 0:D], r_sp[:, 0:NT], "r_ss")
        mulred(x_sp_r, NT, wr_sp[:, D:], r_sp[:, NT:], "r_st")
        r_tm = const.tile([128, 2 * TT], F32)
        mulred(x_tm_r, TT, wr_tm[:, 0:D], r_tm[:, 0:TT], "r_ts")
        mulred(x_tm_r, TT, wr_tm[:, D:], r_tm[:, TT:], "r_tt")

        # ---- main loop: 10 output blocks of 256 rows ----------------------
        def block(k, row0, col, rst, nt, g):
            tmp = tmpp.tile([128, 2 * W], F32, name=f"tmp{k}", tag="tmp")
            ot = outp.tile([128, 2 * W], F16, name=f"ot{k}", tag="ot")
            for h in range(2):          # row 2g+h on partition p
                o = h * W
                rn = rst[:, 2 * g + h : 2 * g + h + 1]
                rt = rst[:, nt + 2 * g + h : nt + 2 * g + h + 1]
                nc.vector.tensor_scalar(
                    tmp[:, o : o + N], col[:, 0:N], rn, 0.0, OP.add, OP.max
                )
                nc.vector.tensor_scalar(
                    tmp[:, o + N : o + W], col[:, N:W], rt, 0.0, OP.add, OP.max
                )
            nc.scalar.activation(ot[:], tmp[:], AF.Tanh)
            nc.sync.dma_start(
                adj[row0 : row0 + 256, :].rearrange("(p t) w -> p (t w)", p=128),
                ot[:],
            )

        for g in range(GN):
            block(g, 256 * g, col_sp, r_sp, NT, g)
        for g in range(GT):
            block(GN + g, N + 256 * g, col_tm, r_tm, TT, g)


def build_nc(num_devices=8):
    nc = bacc.Bacc(
        "TRN2",
        target_bir_lowering=False,
        debug=False,
        enable_asserts=True,
        num_devices=num_devices,
    )
    sp = nc.dram_tensor("spatial_nodes", (N, D), F32, kind="ExternalInput").ap()
    tm = nc.dram_tensor("temporal_nodes", (T, D), F32, kind="ExternalInput").ap()
    ws = {}
    for nm in QUADS:
        ws[f"w_{nm}"] = nc.dram_tensor(f"w_{nm}", (2 * D,), F32, kind="ExternalInput").ap()
        ws[f"b_{nm}"] = nc.dram_tensor(f"b_{nm}", (1,), F32, kind="ExternalInput").ap()
    scr = {
        "sp": nc.dram_tensor("scr_sp", (W,), F32, kind="Internal").ap(),
        "tm": nc.dram_tensor("scr_tm", (W,), F32, kind="Internal").ap(),
    }
    adj = nc.dram_tensor("adj", (W, W), F16, kind="ExternalOutput").ap()

    with tile.TileContext(nc) as tc:
        _emit(tc, sp, tm, ws, scr, adj)
    nc.compile()
    return nc


def make_in_maps(inputs):
    in_maps = []
    for b in range(B):
        m = {
            "spatial_nodes": np.ascontiguousarray(inputs["spatial_nodes"][b], np.float32),
            "temporal_nodes": np.ascontiguousarray(inputs["temporal_nodes"][b], np.float32),
        }
        for nm in QUADS:
            m[f"w_{nm}"] = np.ascontiguousarray(inputs[f"w_{nm}"], np.float32)
            m[f"b_{nm}"] = np.ascontiguousarray(inputs[f"b_{nm}"], np.float32)
        in_maps.append(m)
    return in_maps


_NC = {}


def run(inputs, trace=False, trace_cores=None):
    if 8 not in _NC:
        _NC[8] = build_nc(8)
    res = run_bass_kernel_spmd(
        _NC[8], make_in_maps(inputs), core_ids=list(range(B)), trace=trace,
        trace_cores=trace_cores,
    )
    out = np.stack(
        [res.results[i]["adj"].astype(np.float32) for i in range(B)], axis=0
    )
    return out, res


def kernel(**inputs) -> np.ndarray:
    out, _ = run(inputs, trace=False)
    return out


# revision 13
# speedup vs baseline: 1.4173x; 1.2805x over previous
"""Trainium2 Bass kernel for the MLPConstructor2 adjacency problem.

Computes, per batch b (one NeuronCore each, 8-way data parallel over B):
    adj[i, j] = tanh(relu(x1_i @ w1 + x2_j @ w2 + b))
for the four (spatial/temporal) quadrants of a (2560, 2560) output.

v2 design (ACT-bound, ~46us ScalarE floor):
- Output is stored as fp16 (tolerance is 2e-2; fp16 rounding adds ~5e-4),
  halving the HBM store traffic to 13.1 MB/core.
- Per 256-row block: VectorE computes max(col + row_scalar, 0) with ONE
  fused tensor_scalar (add, max) per quadrant range into an fp32 tmp tile;
  ScalarE then runs a single TANH over the whole [128, 5120] block
  (fp32 in -> fp16 out), and one 1.31 MB store writes 256 rows
  (partition p holds rows 2p, 2p+1 -> 10 KB contiguous per partition).
- tanh(relu(x)) == relu(tanh(x)); we apply relu first so the tanh is the
  last op and its fp16 output stores directly.
- Col-side stats per quadrant pair are computed in (p t) layout, stored
  once to a DRAM scratch in [128, 20]-interleaved order, and broadcast
  back to all partitions with step-0 partition APs (pure DMA broadcast).
- A dummy 1-elem tanh at t=0 pulls the ACT_TABLE_LOAD off the critical
  path; weight/bias broadcasts ride the otherwise-idle GpSimd (SWDGE)
  queue so ScalarE does nothing but TANH.
"""

import numpy as np
from contextlib import ExitStack

import concourse.bacc as bacc
import concourse.mybir as mybir
import concourse.tile as tile
from concourse.bass_utils import run_bass_kernel_spmd

B, N, T, D = 8, 2048, 512, 32
W = N + T                     # 2560
NT, TT = N // 128, T // 128   # 16, 4 col-stat slots
GN, GT = N // 256, T // 256   # 8, 2 output row blocks (256 rows each)
F32 = mybir.dt.float32
F16 = mybir.dt.float16
QUADS = ("ss", "st", "ts", "tt")


def _emit(tc, sp, tm, ws, scr, adj):
    nc = tc.nc
    AF = mybir.ActivationFunctionType
    OP = mybir.AluOpType
    with ExitStack() as ctx:
        ctx.enter_context(nc.allow_low_precision(
            reason="fp16 intermediates; tolerance is 2e-2, fp16 adds ~1e-3"
        ))
        const = ctx.enter_context(tc.tile_pool(name="const", bufs=1))
        tmpp = ctx.enter_context(tc.tile_pool(name="tmpp", bufs=2))
        outp = ctx.enter_context(tc.tile_pool(name="outp", bufs=3))

        # ---- ACT table-load warmup: first tanh pays ~2.7us once ----------
        warm = const.tile([128, 1], F32)
        nc.vector.memset(warm[:], 0.0)
        nc.scalar.activation(warm[:], warm[:], AF.Tanh)

        # ---- stage inputs -------------------------------------------------
        # (p t): row p*nt+t at [p, t*D:(t+1)*D] -- for col-side stats
        x_sp_pt = const.tile([128, NT * D], F32)
        nc.sync.dma_start(x_sp_pt[:], sp.rearrange("(p t) d -> p t d", p=128))
        x_tm_pt = const.tile([128, TT * D], F32)
        nc.sync.dma_start(x_tm_pt[:], tm.rearrange("(p t) d -> p t d", p=128))
        # (g h p): row g*256 + h*128 + p at slot 2g+h -- row-side stats, so a
        # 256-row block is two contiguous 128-row halves (splittable stores).
        x_sp_r = const.tile([128, NT * D], F32)
        nc.sync.dma_start(
            x_sp_r[:].rearrange("p (g h d) -> p g h d", g=GN, h=2),
            sp.rearrange("(g h p) d -> p g h d", g=GN, p=128),
        )
        x_tm_r = const.tile([128, TT * D], F32)
        nc.sync.dma_start(
            x_tm_r[:].rearrange("p (g h d) -> p g h d", g=GT, h=2),
            tm.rearrange("(g h p) d -> p g h d", g=GT, p=128),
        )

        # broadcast weights straight from DRAM with step-0 partition APs.
        # One DMA per weight tensor ([128, 2D]: row half | col half), issued
        # on the Scalar HWDGE ring, which is idle during the prologue.
        # SWDGE (gpsimd) is ~10x slower here; Sync must stay free for the
        # stat scratch round-trip.
        wt = {}
        for nm in QUADS:
            t = const.tile([128, 2 * D], F32, name=f"w_{nm}", tag=f"w_{nm}")
            nc.scalar.dma_start(t[:], ws[f"w_{nm}"].unsqueeze(0).broadcast_to((128, 2 * D)))
            wt[nm] = t
        bb = const.tile([128, 4], F32)   # b_ss, b_st, b_ts, b_tt broadcast
        for j, nm in enumerate(QUADS):
            nc.scalar.dma_start(
                bb[:, j : j + 1], ws[f"b_{nm}"].unsqueeze(0).broadcast_to((128, 1))
            )

        # ---- stats on VectorE: mul + reduce over D ------------------------
        def mulred(x, nt, w, dst, name):
            prod = const.tile([128, nt * D], F32, name=f"prod_{name}", tag="prod")
            x3 = x[:].rearrange("p (t d) -> p t d", t=nt)
            p3 = prod[:].rearrange("p (t d) -> p t d", t=nt)
            w3 = w.unsqueeze(1).broadcast_to((128, nt, D))
            nc.vector.tensor_tensor(p3, x3, w3, OP.mult)
            nc.vector.tensor_reduce(dst, p3, axis=mybir.AxisListType.X, op=OP.add)

        # col stats for one output half (ss+st -> col_sp, ts+tt -> col_tm):
        # [128, 20] stat tile (16 spatial slots + 4 temporal, quadrant biases
        # folded in), stored to scr keeping DRAM contiguous per range so the
        # partition-broadcast reload reads ONE contiguous chunk per partition
        # (128 big descriptors, not 16k tiny ones).
        def col_half(wsp_sl, wtm_sl, b_sp, b_tm, scr_t, name):
            cst = const.tile([128, NT + TT], F16, name=f"cst_{name}", tag=f"cst_{name}")
            col = const.tile([128, W], F16, name=f"col_{name}", tag=f"col_{name}")
            mulred(x_sp_pt, NT, wsp_sl, cst[:, 0:NT], f"c{name}s")
            nc.vector.tensor_scalar_add(cst[:, 0:NT], cst[:, 0:NT], b_sp)
            nc.sync.dma_start(
                scr_t[0:N].rearrange("(p j) -> p j", p=128), cst[:, 0:NT]
            )
            nc.sync.dma_start(
                col[:, 0:N], scr_t[0:N].unsqueeze(0).broadcast_to((128, N))
            )
            mulred(x_tm_pt, TT, wtm_sl, cst[:, NT:], f"c{name}t")
            nc.vector.tensor_scalar_add(cst[:, NT:], cst[:, NT:], b_tm)
            nc.sync.dma_start(
                scr_t[N:W].rearrange("(p j) -> p j", p=128), cst[:, NT:]
            )
            nc.sync.dma_start(
                col[:, N:W], scr_t[N:W].unsqueeze(0).broadcast_to((128, T))
            )
            return col

        col_sp = col_half(wt["ss"][:, D:], wt["st"][:, D:], bb[:, 0:1], bb[:, 1:2],
                          scr["sp"], "sp")
        col_tm = col_half(wt["ts"][:, D:], wt["tt"][:, D:], bb[:, 2:3], bb[:, 3:4],
                          scr["tm"], "tm")

        # row stats: slots [a_q1 (0:nt), a_q2 (nt:2nt)], block g rows = 2g, 2g+1
        r_sp = const.tile([128, 2 * NT], F32)
        mulred(x_sp_r, NT, wt["ss"][:, 0:D], r_sp[:, 0:NT], "r_ss")
        mulred(x_sp_r, NT, wt["st"][:, 0:D], r_sp[:, NT:], "r_st")
        r_tm = const.tile([128, 2 * TT], F32)
        mulred(x_tm_r, TT, wt["ts"][:, 0:D], r_tm[:, 0:TT], "r_ts")
        mulred(x_tm_r, TT, wt["tt"][:, 0:D], r_tm[:, TT:], "r_tt")

        # ---- main loop: 10 output blocks of 256 rows ----------------------
        # half h of block g = output rows [row0 + 128h, row0 + 128h + 128),
        # partition p -> row row0 + 128h + p (row scalar = slot 2g+h).
        def block(k, row0, col, rst, nt, g, split=False):
            tmp = tmpp.tile([128, 2 * W], F16, name=f"tmp{k}", tag="tmp")
            ot = outp.tile([128, 2 * W], F16, name=f"ot{k}", tag="ot")
            for h in range(2):
                o = h * W
                rn = rst[:, 2 * g + h : 2 * g + h + 1]
                rt = rst[:, nt + 2 * g + h : nt + 2 * g + h + 1]
                nc.vector.tensor_scalar(
                    tmp[:, o : o + N], col[:, 0:N], rn, 0.0, OP.add, OP.max
                )
                nc.vector.tensor_scalar(
                    tmp[:, o + N : o + W], col[:, N:W], rt, 0.0, OP.add, OP.max
                )
                if split:
                    nc.scalar.activation(ot[:, o : o + W], tmp[:, o : o + W], AF.Tanh)
                    nc.sync.dma_start(
                        adj[row0 + 128 * h : row0 + 128 * h + 128, :],
                        ot[:, o : o + W],
                    )
            if not split:
                nc.scalar.activation(ot[:], tmp[:], AF.Tanh)
                nc.sync.dma_start(
                    adj[row0 : row0 + 256, :].rearrange("(h p) w -> p h w", p=128),
                    ot[:].rearrange("p (h w) -> p h w", h=2),
                )

        for g in range(GN):
            block(g, 256 * g, col_sp, r_sp, NT, g)
        for g in range(GT):
            block(GN + g, N + 256 * g, col_tm, r_tm, TT, g,
                  split=(g == GT - 1))


def build_nc(num_devices=8):
    nc = bacc.Bacc(
        "TRN2",
        target_bir_lowering=False,
        debug=False,
        enable_asserts=True,
        num_devices=num_devices,
    )
    sp = nc.dram_tensor("spatial_nodes", (N, D), F32, kind="ExternalInput").ap()
    tm = nc.dram_tensor("temporal_nodes", (T, D), F32, kind="ExternalInput").ap()
    ws = {}
    for nm in QUADS:
        ws[f"w_{nm}"] = nc.dram_tensor(f"w_{nm}", (2 * D,), F32, kind="ExternalInput").ap()
        ws[f"b_{nm}"] = nc.dram_tensor(f"b_{nm}", (1,), F32, kind="ExternalInput").ap()
    scr = {
        "sp": nc.dram_tensor("scr_sp", (W,), F16, kind="Internal").ap(),
        "tm": nc.dram_tensor("scr_tm", (W,), F16, kind="Internal").ap(),
    }
    adj = nc.dram_tensor("adj", (W, W), F16, kind="ExternalOutput").ap()

    with tile.TileContext(nc) as tc:
        _emit(tc, sp, tm, ws, scr, adj)
    nc.compile()
    return nc


def make_in_maps(inputs):
    in_maps = []
    for b in range(B):
        m = {
            "spatial_nodes": np.ascontiguousarray(inputs["spatial_nodes"][b], np.float32),
            "temporal_nodes": np.ascontiguousarray(inputs["temporal_nodes"][b], np.float32),
        }
        for nm in QUADS:
            m[f"w_{nm}"] = np.ascontiguousarray(inputs[f"w_{nm}"], np.float32)
            m[f"b_{nm}"] = np.ascontiguousarray(inputs[f"b_{nm}"], np.float32)
        in_maps.append(m)
    return in_maps


_NC = {}


def run(inputs, trace=False, trace_cores=None):
    if 8 not in _NC:
        _NC[8] = build_nc(8)
    res = run_bass_kernel_spmd(
        _NC[8], make_in_maps(inputs), core_ids=list(range(B)), trace=trace,
        trace_cores=trace_cores,
    )
    out = np.stack(
        [res.results[i]["adj"].astype(np.float32) for i in range(B)], axis=0
    )
    return out, res


def kernel(**inputs) -> np.ndarray:
    out, _ = run(inputs, trace=False)
    return out


# revision 15
# speedup vs baseline: 1.4663x; 1.0346x over previous
"""Trainium2 Bass kernel for the MLPConstructor2 adjacency problem.

Computes, per batch b (one NeuronCore each, 8-way data parallel over B):
    adj[i, j] = tanh(relu(x1_i @ w1 + x2_j @ w2 + b))
for the four (spatial/temporal) quadrants of a (2560, 2560) output.

v5 design (ACT-bound, ~46us ScalarE floor):
- Output is stored as fp16 (tolerance is 2e-2; fp16 adds ~1e-3), halving
  the HBM store traffic to 13.1 MB/core. tanh(relu(x)) == relu(tanh(x)),
  so relu runs first (fused on VectorE) and tanh's fp16 result stores
  directly.
- x is staged ONCE per node set, in (p t) layout; both the col stats and
  the row scalars come from it. Output blocks are row-strided to match:
  block t covers rows {16p+t} u {16p+t+8}, so the per-partition row
  scalar is stat slot t (resp. t+8) with no second staging layout.
- Per block: 4 fused VectorE tensor_scalar (add row scalar, max 0) fill
  an fp16 tmp [128, 5120]; ScalarE runs ONE tanh over it (fp16 in/out);
  one 1.31 MB store writes the 256 strided rows (256 x 5120 B descs).
- All weights+biases arrive pre-packed in one "wpack" input (host-side
  concat), loaded with a single partition-broadcast DMA. Quadrant biases
  fold into the row stats (off the col critical path).
- Col stats round-trip through a DRAM scratch and return partition-
  broadcast; the spatial chain rides the Sync ring, the temporal chain
  the Scalar ring, so the two overlap. A dummy tanh at t=0 pulls the
  ACT_TABLE_LOAD off the critical path.
"""

import numpy as np
from contextlib import ExitStack

import concourse.bacc as bacc
import concourse.mybir as mybir
import concourse.tile as tile
from concourse.bass_utils import run_bass_kernel_spmd

B, N, T, D = 8, 2048, 512, 32
W = N + T                     # 2560
NT, TT = N // 128, T // 128   # 16, 4 stat slots per partition
F32 = mybir.dt.float32
F16 = mybir.dt.float16
QUADS = ("ss", "st", "ts", "tt")


def _emit(tc, sp, tm, wp_in, scr, adj):
    nc = tc.nc
    AF = mybir.ActivationFunctionType
    OP = mybir.AluOpType
    with ExitStack() as ctx:
        ctx.enter_context(nc.allow_low_precision(
            reason="fp16 intermediates; tolerance is 2e-2, fp16 adds ~1e-3"
        ))
        const = ctx.enter_context(tc.tile_pool(name="const", bufs=1))
        tmpp = ctx.enter_context(tc.tile_pool(name="tmpp", bufs=2))
        outp = ctx.enter_context(tc.tile_pool(name="outp", bufs=3))

        # ---- one broadcast load for all weights + biases ------------------
        # wpack = [w_ss | w_st | w_ts | w_tt | b_ss b_st b_ts b_tt] (260 f32)
        wp = const.tile([128, 260], F32)
        nc.scalar.dma_start(wp[:], wp_in.unsqueeze(0).broadcast_to((128, 260)))

        def w_row(q):  # first half of w_q: row-side weights
            return wp[:, 64 * q : 64 * q + D]

        def w_col(q):  # second half: col-side weights
            return wp[:, 64 * q + D : 64 * q + 2 * D]

        def b_q(q):
            return wp[:, 256 + q : 257 + q]

        # ---- ACT table-load warmup (overlaps the wpack transfer) ----------
        warm = const.tile([128, 1], F32)
        nc.vector.memset(warm[:], 0.0)
        nc.scalar.activation(warm[:], warm[:], AF.Tanh)

        # ---- stage inputs, (p t) layout: row p*nt+t at [p, t*D:(t+1)*D] ---
        x_sp = const.tile([128, NT * D], F32)
        nc.sync.dma_start(x_sp[:], sp.rearrange("(p t) d -> p t d", p=128))
        x_tm = const.tile([128, TT * D], F32)
        nc.sync.dma_start(x_tm[:], tm.rearrange("(p t) d -> p t d", p=128))

        # ---- stats on VectorE: mul + reduce over D ------------------------
        def mulred(x, nt, w, dst, name):
            prod = const.tile([128, nt * D], F32, name=f"prod_{name}", tag="prod")
            x3 = x[:].rearrange("p (t d) -> p t d", t=nt)
            p3 = prod[:].rearrange("p (t d) -> p t d", t=nt)
            w3 = w.unsqueeze(1).broadcast_to((128, nt, D))
            nc.vector.tensor_tensor(p3, x3, w3, OP.mult)
            nc.vector.tensor_reduce(dst, p3, axis=mybir.AxisListType.X, op=OP.add)

        # col stats for one output half (ss+st -> col_sp, ts+tt -> col_tm):
        # fp16 [128, W] rebuilt via DRAM scratch + partition-broadcast.
        # dma_eng picks the HWDGE ring so the two halves' chains overlap.
        def col_half(q_sp, q_tm, scr_t, dma_eng, name):
            cst = const.tile([128, NT + TT], F16, name=f"cst_{name}", tag=f"cst_{name}")
            col = const.tile([128, W], F16, name=f"col_{name}", tag=f"col_{name}")
            mulred(x_sp, NT, w_col(q_sp), cst[:, 0:NT], f"c{name}s")
            dma_eng.dma_start(
                scr_t[0:N].rearrange("(p j) -> p j", p=128), cst[:, 0:NT]
            )
            dma_eng.dma_start(
                col[:, 0:N], scr_t[0:N].unsqueeze(0).broadcast_to((128, N))
            )
            mulred(x_tm, TT, w_col(q_tm), cst[:, NT:], f"c{name}t")
            dma_eng.dma_start(
                scr_t[N:W].rearrange("(p j) -> p j", p=128), cst[:, NT:]
            )
            dma_eng.dma_start(
                col[:, N:W], scr_t[N:W].unsqueeze(0).broadcast_to((128, T))
            )
            return col

        col_sp = col_half(0, 1, scr["sp"], nc.sync, "sp")    # w_ss2, w_st2
        col_tm = col_half(2, 3, scr["tm"], nc.scalar, "tm")  # w_ts2, w_tt2

        # row stats (slot t = row p*nt + t), quadrant biases folded in:
        # r_sp = [a_ss + b_ss | a_st + b_st], r_tm = [a_ts + b_ts | a_tt + b_tt]
        r_sp = const.tile([128, 2 * NT], F32)
        mulred(x_sp, NT, w_row(0), r_sp[:, 0:NT], "r_ss")
        nc.vector.tensor_scalar_add(r_sp[:, 0:NT], r_sp[:, 0:NT], b_q(0))
        mulred(x_sp, NT, w_row(1), r_sp[:, NT:], "r_st")
        nc.vector.tensor_scalar_add(r_sp[:, NT:], r_sp[:, NT:], b_q(1))
        r_tm = const.tile([128, 2 * TT], F32)
        mulred(x_tm, TT, w_row(2), r_tm[:, 0:TT], "r_ts")
        nc.vector.tensor_scalar_add(r_tm[:, 0:TT], r_tm[:, 0:TT], b_q(2))
        mulred(x_tm, TT, w_row(3), r_tm[:, TT:], "r_tt")
        nc.vector.tensor_scalar_add(r_tm[:, TT:], r_tm[:, TT:], b_q(3))

        # ---- main loop: strided 256-row blocks -----------------------------
        # spatial block t (t=0..7): rows {16p+t} (h=0) and {16p+t+8} (h=1)
        # temporal block t (t=0..1): rows 2048 + {4p+t} and 2048 + {4p+t+2}
        def block(k, t, base, nt, col, rst, hs, split=False):
            tmp = tmpp.tile([128, 2 * W], F16, name=f"tmp{k}", tag="tmp")
            ot = outp.tile([128, 2 * W], F16, name=f"ot{k}", tag="ot")
            quad = adj[base : base + 128 * nt, :]
            for h in range(2):
                o = h * W
                s = t + h * hs
                rn = rst[:, s : s + 1]
                rt = rst[:, nt + s : nt + s + 1]
                nc.vector.tensor_scalar(
                    tmp[:, o : o + N], col[:, 0:N], rn, 0.0, OP.add, OP.max
                )
                nc.vector.tensor_scalar(
                    tmp[:, o + N : o + W], col[:, N:W], rt, 0.0, OP.add, OP.max
                )
                if split:
                    nc.scalar.activation(ot[:, o : o + W], tmp[:, o : o + W], AF.Tanh)
                    nc.sync.dma_start(
                        quad.rearrange("(p r) w -> p r w", p=128)[:, s : s + 1, :],
                        ot[:, o : o + W].rearrange("p (r w) -> p r w", r=1),
                    )
            if not split:
                nc.scalar.activation(ot[:], tmp[:], AF.Tanh)
                # partition p -> rows base + nt*p + t and base + nt*p + t + hs
                nc.sync.dma_start(
                    quad.rearrange("(p g r) w -> p g r w", p=128, g=2)[
                        :, :, t : t + 1, :
                    ],
                    ot[:].rearrange("p (g w) -> p g w", g=2).unsqueeze(2),
                )

        for t in range(NT // 2):
            block(t, t, 0, NT, col_sp, r_sp, NT // 2)
        for t in range(TT // 2):
            block(8 + t, t, N, TT, col_tm, r_tm, TT // 2,
                  split=(t == TT // 2 - 1))


def build_nc(num_devices=8):
    nc = bacc.Bacc(
        "TRN2",
        target_bir_lowering=False,
        debug=False,
        enable_asserts=True,
        num_devices=num_devices,
    )
    sp = nc.dram_tensor("spatial_nodes", (N, D), F32, kind="ExternalInput").ap()
    tm = nc.dram_tensor("temporal_nodes", (T, D), F32, kind="ExternalInput").ap()
    wp = nc.dram_tensor("wpack", (260,), F32, kind="ExternalInput").ap()
    scr = {
        "sp": nc.dram_tensor("scr_sp", (W,), F16, kind="Internal").ap(),
        "tm": nc.dram_tensor("scr_tm", (W,), F16, kind="Internal").ap(),
    }
    adj = nc.dram_tensor("adj", (W, W), F16, kind="ExternalOutput").ap()

    with tile.TileContext(nc) as tc:
        _emit(tc, sp, tm, wp, scr, adj)
    nc.compile()
    return nc


def make_in_maps(inputs):
    wpack = np.concatenate(
        [np.asarray(inputs[f"w_{nm}"], np.float32).reshape(-1) for nm in QUADS]
        + [np.asarray(inputs[f"b_{nm}"], np.float32).reshape(-1) for nm in QUADS]
    )
    in_maps = []
    for b in range(B):
        m = {
            "spatial_nodes": np.ascontiguousarray(inputs["spatial_nodes"][b], np.float32),
            "temporal_nodes": np.ascontiguousarray(inputs["temporal_nodes"][b], np.float32),
            "wpack": wpack,
        }
        in_maps.append(m)
    return in_maps


_NC = {}


def run(inputs, trace=False, trace_cores=None):
    if 8 not in _NC:
        _NC[8] = build_nc(8)
    res = run_bass_kernel_spmd(
        _NC[8], make_in_maps(inputs), core_ids=list(range(B)), trace=trace,
        trace_cores=trace_cores,
    )
    out = np.stack(
        [res.results[i]["adj"].astype(np.float32) for i in range(B)], axis=0
    )
    return out, res


def kernel(**inputs) -> np.ndarray:
    out, _ = run(inputs, trace=False)
    return out


# revision 19
# speedup vs baseline: 1.4694x; 1.0021x over previous
"""Trainium2 Bass kernel for the MLPConstructor2 adjacency problem.

Computes, per batch b (one NeuronCore each, 8-way data parallel over B):
    adj[i, j] = tanh(relu(x1_i @ w1 + x2_j @ w2 + b))
for the four (spatial/temporal) quadrants of a (2560, 2560) output.

v5 design (ACT-bound, ~46us ScalarE floor):
- Output is stored as fp16 (tolerance is 2e-2; fp16 adds ~1e-3), halving
  the HBM store traffic to 13.1 MB/core. tanh(relu(x)) == relu(tanh(x)),
  so relu runs first (fused on VectorE) and tanh's fp16 result stores
  directly.
- x is staged ONCE per node set, in (p t) layout; both the col stats and
  the row scalars come from it. Output blocks are row-strided to match:
  block t covers rows {16p+t} u {16p+t+8}, so the per-partition row
  scalar is stat slot t (resp. t+8) with no second staging layout.
- Per block: 4 fused VectorE tensor_scalar (add row scalar, max 0) fill
  an fp16 tmp [128, 5120]; ScalarE runs ONE tanh over it (fp16 in/out);
  one 1.31 MB store writes the 256 strided rows (256 x 5120 B descs).
- All weights+biases arrive pre-packed in one "wpack" input (host-side
  concat), loaded with a single partition-broadcast DMA. Quadrant biases
  fold into the row stats (off the col critical path).
- Col stats round-trip through a DRAM scratch and return partition-
  broadcast; the spatial chain rides the Sync ring, the temporal chain
  the Scalar ring, so the two overlap. A dummy tanh at t=0 pulls the
  ACT_TABLE_LOAD off the critical path.
"""

import numpy as np
from contextlib import ExitStack

import concourse.bacc as bacc
import concourse.mybir as mybir
import concourse.tile as tile
from concourse.bass_utils import run_bass_kernel_spmd

B, N, T, D = 8, 2048, 512, 32
W = N + T                     # 2560
NT, TT = N // 128, T // 128   # 16, 4 stat slots per partition
F32 = mybir.dt.float32
F16 = mybir.dt.float16
QUADS = ("ss", "st", "ts", "tt")


def _emit(tc, sp, tm, wp_in, scr, adj):
    nc = tc.nc
    AF = mybir.ActivationFunctionType
    OP = mybir.AluOpType
    with ExitStack() as ctx:
        ctx.enter_context(nc.allow_low_precision(
            reason="fp16 intermediates; tolerance is 2e-2, fp16 adds ~1e-3"
        ))
        const = ctx.enter_context(tc.tile_pool(name="const", bufs=1))
        tmpp = ctx.enter_context(tc.tile_pool(name="tmpp", bufs=2))
        outp = ctx.enter_context(tc.tile_pool(name="outp", bufs=3))

        # ---- one broadcast load for all weights + biases ------------------
        # wpack = [w_ss | w_st | w_ts | w_tt | b_ss b_st b_ts b_tt] (260 f32)
        wp = const.tile([128, 260], F32)
        nc.scalar.dma_start(wp[:], wp_in.unsqueeze(0).broadcast_to((128, 260)))

        def w_row(q):  # first half of w_q: row-side weights
            return wp[:, 64 * q : 64 * q + D]

        def w_col(q):  # second half: col-side weights
            return wp[:, 64 * q + D : 64 * q + 2 * D]

        def b_q(q):
            return wp[:, 256 + q : 257 + q]

        # ---- ACT table-load warmup (overlaps the wpack transfer) ----------
        warm = const.tile([128, 1], F32)
        nc.vector.memset(warm[:], 0.0)
        nc.scalar.activation(warm[:], warm[:], AF.Tanh)

        # ---- stage inputs, (p t) layout: row p*nt+t at [p, t*D:(t+1)*D] ---
        # x_tm first: it is 4x smaller and gates the quick T-part stat chain.
        x_tm = const.tile([128, TT * D], F32)
        nc.sync.dma_start(x_tm[:], tm.rearrange("(p t) d -> p t d", p=128))
        x_sp = const.tile([128, NT * D], F32)
        nc.sync.dma_start(x_sp[:], sp.rearrange("(p t) d -> p t d", p=128))

        # ---- stats on VectorE: mul + reduce over D ------------------------
        def mulred(x, nt, w, dst, name):
            prod = const.tile([128, nt * D], F32, name=f"prod_{name}", tag="prod")
            x3 = x[:].rearrange("p (t d) -> p t d", t=nt)
            p3 = prod[:].rearrange("p (t d) -> p t d", t=nt)
            w3 = w.unsqueeze(1).broadcast_to((128, nt, D))
            nc.vector.tensor_tensor(p3, x3, w3, OP.mult)
            nc.vector.tensor_reduce(dst, p3, axis=mybir.AxisListType.X, op=OP.add)

        # col stats for one output half (ss+st -> col_sp, ts+tt -> col_tm):
        # fp16 [128, W] rebuilt via DRAM scratch + partition-broadcast.
        # dma_eng picks the HWDGE ring so the two halves' chains overlap.
        # col stats for one output half (ss+st -> col_sp, ts+tt -> col_tm):
        # fp16 [128, W] rebuilt via DRAM scratch + partition-broadcast.
        # The small T-part (from x_tm) runs its whole chain first so its
        # two DMA-hop latencies overlap the larger N-part's compute.
        # dma_eng picks the HWDGE ring so the two halves' chains overlap.
        def col_half(q_sp, q_tm, scr_t, dma_eng, name):
            cst = const.tile([128, NT + TT], F16, name=f"cst_{name}", tag=f"cst_{name}")
            col = const.tile([128, W], F16, name=f"col_{name}", tag=f"col_{name}")
            mulred(x_tm, TT, w_col(q_tm), cst[:, NT:], f"c{name}t")
            dma_eng.dma_start(
                scr_t[N:W].rearrange("(p j) -> p j", p=128), cst[:, NT:]
            )
            dma_eng.dma_start(
                col[:, N:W], scr_t[N:W].unsqueeze(0).broadcast_to((128, T))
            )
            mulred(x_sp, NT, w_col(q_sp), cst[:, 0:NT], f"c{name}s")
            dma_eng.dma_start(
                scr_t[0:N].rearrange("(p j) -> p j", p=128), cst[:, 0:NT]
            )
            dma_eng.dma_start(
                col[:, 0:N], scr_t[0:N].unsqueeze(0).broadcast_to((128, N))
            )
            return col

        col_sp = col_half(0, 1, scr["sp"], nc.sync, "sp")    # w_ss2, w_st2

        # row stats (slot t = row p*nt + t), quadrant biases folded in:
        # r_sp = [a_ss + b_ss | a_st + b_st], r_tm = [a_ts + b_ts | a_tt + b_tt]
        r_sp = const.tile([128, 2 * NT], F32)
        mulred(x_sp, NT, w_row(0), r_sp[:, 0:NT], "r_ss")
        nc.vector.tensor_scalar_add(r_sp[:, 0:NT], r_sp[:, 0:NT], b_q(0))
        mulred(x_sp, NT, w_row(1), r_sp[:, NT:], "r_st")
        nc.vector.tensor_scalar_add(r_sp[:, NT:], r_sp[:, NT:], b_q(1))

        col_tm = col_half(2, 3, scr["tm"], nc.scalar, "tm")  # w_ts2, w_tt2

        r_tm = const.tile([128, 2 * TT], F32)
        mulred(x_tm, TT, w_row(2), r_tm[:, 0:TT], "r_ts")
        nc.vector.tensor_scalar_add(r_tm[:, 0:TT], r_tm[:, 0:TT], b_q(2))
        mulred(x_tm, TT, w_row(3), r_tm[:, TT:], "r_tt")
        nc.vector.tensor_scalar_add(r_tm[:, TT:], r_tm[:, TT:], b_q(3))

        # ---- main loop: strided 256-row blocks -----------------------------
        # spatial block t (t=0..7): rows {16p+t} (h=0) and {16p+t+8} (h=1)
        # temporal block t (t=0..1): rows 2048 + {4p+t} and 2048 + {4p+t+2}
        def block(k, t, base, nt, col, rst, hs, split=False):
            tmp = tmpp.tile([128, 2 * W], F16, name=f"tmp{k}", tag="tmp")
            ot = outp.tile([128, 2 * W], F16, name=f"ot{k}", tag="ot")
            quad = adj[base : base + 128 * nt, :]
            for h in range(2):
                o = h * W
                s = t + h * hs
                rn = rst[:, s : s + 1]
                rt = rst[:, nt + s : nt + s + 1]
                nc.vector.tensor_scalar(
                    tmp[:, o + N : o + W], col[:, N:W], rt, 0.0, OP.add, OP.max
                )
                nc.vector.tensor_scalar(
                    tmp[:, o : o + N], col[:, 0:N], rn, 0.0, OP.add, OP.max
                )
                if split:
                    nc.scalar.activation(ot[:, o : o + W], tmp[:, o : o + W], AF.Tanh)
                    nc.sync.dma_start(
                        quad.rearrange("(p r) w -> p r w", p=128)[:, s : s + 1, :],
                        ot[:, o : o + W].rearrange("p (r w) -> p r w", r=1),
                    )
            if not split:
                nc.scalar.activation(ot[:], tmp[:], AF.Tanh)
                # partition p -> rows base + nt*p + t and base + nt*p + t + hs
                nc.sync.dma_start(
                    quad.rearrange("(p g r) w -> p g r w", p=128, g=2)[
                        :, :, t : t + 1, :
                    ],
                    ot[:].rearrange("p (g w) -> p g w", g=2).unsqueeze(2),
                )

        for t in range(NT // 2):
            block(t, t, 0, NT, col_sp, r_sp, NT // 2, split=(t == 0))
        for t in range(TT // 2):
            block(8 + t, t, N, TT, col_tm, r_tm, TT // 2,
                  split=(t == TT // 2 - 1))


def build_nc(num_devices=8):
    nc = bacc.Bacc(
        "TRN2",
        target_bir_lowering=False,
        debug=False,
        enable_asserts=True,
        num_devices=num_devices,
    )
    sp = nc.dram_tensor("spatial_nodes", (N, D), F32, kind="ExternalInput").ap()
    tm = nc.dram_tensor("temporal_nodes", (T, D), F32, kind="ExternalInput").ap()
    wp = nc.dram_tensor("wpack", (260,), F32, kind="ExternalInput").ap()
    scr = {
        "sp": nc.dram_tensor("scr_sp", (W,), F16, kind="Internal").ap(),
        "tm": nc.dram_tensor("scr_tm", (W,), F16, kind="Internal").ap(),
    }
    adj = nc.dram_tensor("adj", (W, W), F16, kind="ExternalOutput").ap()

    with tile.TileContext(nc) as tc:
        _emit(tc, sp, tm, wp, scr, adj)
    nc.compile()
    return nc


def make_in_maps(inputs):
    wpack = np.concatenate(
        [np.asarray(inputs[f"w_{nm}"], np.float32).reshape(-1) for nm in QUADS]
        + [np.asarray(inputs[f"b_{nm}"], np.float32).reshape(-1) for nm in QUADS]
    )
    in_maps = []
    for b in range(B):
        m = {
            "spatial_nodes": np.ascontiguousarray(inputs["spatial_nodes"][b], np.float32),
            "temporal_nodes": np.ascontiguousarray(inputs["temporal_nodes"][b], np.float32),
            "wpack": wpack,
        }
        in_maps.append(m)
    return in_maps


_NC = {}


def run(inputs, trace=False, trace_cores=None):
    if 8 not in _NC:
        _NC[8] = build_nc(8)
    res = run_bass_kernel_spmd(
        _NC[8], make_in_maps(inputs), core_ids=list(range(B)), trace=trace,
        trace_cores=trace_cores,
    )
    out = np.stack(
        [res.results[i]["adj"].astype(np.float32) for i in range(B)], axis=0
    )
    return out, res


def kernel(**inputs) -> np.ndarray:
    out, _ = run(inputs, trace=False)
    return out


# revision 20
# speedup vs baseline: 1.4967x; 1.0186x over previous
"""Trainium2 Bass kernel for the MLPConstructor2 adjacency problem.

Computes, per batch b (one NeuronCore each, 8-way data parallel over B):
    adj[i, j] = tanh(relu(x1_i @ w1 + x2_j @ w2 + b))
for the four (spatial/temporal) quadrants of a (2560, 2560) output.

v5 design (ACT-bound, ~46us ScalarE floor):
- Output is stored as fp16 (tolerance is 2e-2; fp16 adds ~1e-3), halving
  the HBM store traffic to 13.1 MB/core. tanh(relu(x)) == relu(tanh(x)),
  so relu runs first (fused on VectorE) and tanh's fp16 result stores
  directly.
- x is staged ONCE per node set, in (p t) layout; both the col stats and
  the row scalars come from it. Output blocks are row-strided to match:
  block t covers rows {16p+t} u {16p+t+8}, so the per-partition row
  scalar is stat slot t (resp. t+8) with no second staging layout.
- Per block: 4 fused VectorE tensor_scalar (add row scalar, max 0) fill
  an fp16 tmp [128, 5120]; ScalarE runs ONE tanh over it (fp16 in/out);
  one 1.31 MB store writes the 256 strided rows (256 x 5120 B descs).
- All weights+biases arrive pre-packed in one "wpack" input (host-side
  concat), loaded with a single partition-broadcast DMA. Quadrant biases
  fold into the row stats (off the col critical path).
- Col stats round-trip through a DRAM scratch and return partition-
  broadcast; the spatial chain rides the Sync ring, the temporal chain
  the Scalar ring, so the two overlap. A dummy tanh at t=0 pulls the
  ACT_TABLE_LOAD off the critical path.
"""

import numpy as np
from contextlib import ExitStack

import concourse.bacc as bacc
import concourse.mybir as mybir
import concourse.tile as tile
from concourse.bass_utils import run_bass_kernel_spmd

B, N, T, D = 8, 2048, 512, 32
W = N + T                     # 2560
NT, TT = N // 128, T // 128   # 16, 4 stat slots per partition
F32 = mybir.dt.float32
F16 = mybir.dt.float16
QUADS = ("ss", "st", "ts", "tt")


def _emit(tc, sp, tm, wp_in, scr, adj):
    nc = tc.nc
    AF = mybir.ActivationFunctionType
    OP = mybir.AluOpType
    with ExitStack() as ctx:
        ctx.enter_context(nc.allow_low_precision(
            reason="fp16 intermediates; tolerance is 2e-2, fp16 adds ~1e-3"
        ))
        const = ctx.enter_context(tc.tile_pool(name="const", bufs=1))
        tmpp = ctx.enter_context(tc.tile_pool(name="tmpp", bufs=2))
        outp = ctx.enter_context(tc.tile_pool(name="outp", bufs=3))

        # ---- one broadcast load for all weights + biases ------------------
        # wpack = [w_ss | w_st | w_ts | w_tt | b_ss b_st b_ts b_tt] (260 f32)
        wp = const.tile([128, 260], F32)
        nc.scalar.dma_start(wp[:], wp_in.unsqueeze(0).broadcast_to((128, 260)))

        def w_row(q):  # first half of w_q: row-side weights
            return wp[:, 64 * q : 64 * q + D]

        def w_col(q):  # second half: col-side weights
            return wp[:, 64 * q + D : 64 * q + 2 * D]

        def b_q(q):
            return wp[:, 256 + q : 257 + q]

        # ---- ACT table-load warmup (overlaps the wpack transfer) ----------
        warm = const.tile([128, 1], F32)
        nc.vector.memset(warm[:], 0.0)
        nc.scalar.activation(warm[:], warm[:], AF.Tanh)

        # ---- stage inputs, (p t) layout: row p*nt+t at [p, t*D:(t+1)*D] ---
        # x_tm first: it is 4x smaller and gates the quick T-part stat chain.
        x_tm = const.tile([128, TT * D], F32)
        nc.sync.dma_start(x_tm[:], tm.rearrange("(p t) d -> p t d", p=128))
        x_sp = const.tile([128, NT * D], F32)
        nc.sync.dma_start(x_sp[:], sp.rearrange("(p t) d -> p t d", p=128))

        # ---- stats on VectorE: mul + reduce over D ------------------------
        def mulred(x, nt, w, dst, name):
            prod = const.tile([128, nt * D], F32, name=f"prod_{name}", tag="prod")
            x3 = x[:].rearrange("p (t d) -> p t d", t=nt)
            p3 = prod[:].rearrange("p (t d) -> p t d", t=nt)
            w3 = w.unsqueeze(1).broadcast_to((128, nt, D))
            nc.vector.tensor_tensor(p3, x3, w3, OP.mult)
            nc.vector.tensor_reduce(dst, p3, axis=mybir.AxisListType.X, op=OP.add)

        # col stats for one output half (ss+st -> col_sp, ts+tt -> col_tm):
        # fp16 [128, W] rebuilt via DRAM scratch + partition-broadcast.
        # dma_eng picks the HWDGE ring so the two halves' chains overlap.
        # col stats for one output half (ss+st -> col_sp, ts+tt -> col_tm):
        # fp16 [128, W] rebuilt via DRAM scratch + partition-broadcast.
        # The small T-part (from x_tm) runs its whole chain first so its
        # two DMA-hop latencies overlap the larger N-part's compute.
        # dma_eng picks the HWDGE ring so the two halves' chains overlap.
        def col_half(q_sp, q_tm, scr_t, dma_eng, name):
            cst = const.tile([128, NT + TT], F16, name=f"cst_{name}", tag=f"cst_{name}")
            col = const.tile([128, W], F16, name=f"col_{name}", tag=f"col_{name}")
            mulred(x_tm, TT, w_col(q_tm), cst[:, NT:], f"c{name}t")
            dma_eng.dma_start(
                scr_t[N:W].rearrange("(p j) -> p j", p=128), cst[:, NT:]
            )
            dma_eng.dma_start(
                col[:, N:W], scr_t[N:W].unsqueeze(0).broadcast_to((128, T))
            )
            mulred(x_sp, NT, w_col(q_sp), cst[:, 0:NT], f"c{name}s")
            dma_eng.dma_start(
                scr_t[0:N].rearrange("(p j) -> p j", p=128), cst[:, 0:NT]
            )
            dma_eng.dma_start(
                col[:, 0:N], scr_t[0:N].unsqueeze(0).broadcast_to((128, N))
            )
            return col

        col_sp = col_half(0, 1, scr["sp"], nc.sync, "sp")    # w_ss2, w_st2

        # row stats (slot t = row p*nt + t), quadrant biases folded in:
        # r_sp = [a_ss + b_ss | a_st + b_st], r_tm = [a_ts + b_ts | a_tt + b_tt]
        r_sp = const.tile([128, 2 * NT], F32)
        mulred(x_sp, NT, w_row(0), r_sp[:, 0:NT], "r_ss")
        nc.vector.tensor_scalar_add(r_sp[:, 0:NT], r_sp[:, 0:NT], b_q(0))
        mulred(x_sp, NT, w_row(1), r_sp[:, NT:], "r_st")
        nc.vector.tensor_scalar_add(r_sp[:, NT:], r_sp[:, NT:], b_q(1))

        # gpsimd (SWDGE) is slower but idle, and col_tm has ~40us of slack;
        # keeping it off Sync/Scalar keeps block 0's TANH path clear.
        col_tm = col_half(2, 3, scr["tm"], nc.gpsimd, "tm")  # w_ts2, w_tt2

        r_tm = const.tile([128, 2 * TT], F32)
        mulred(x_tm, TT, w_row(2), r_tm[:, 0:TT], "r_ts")
        nc.vector.tensor_scalar_add(r_tm[:, 0:TT], r_tm[:, 0:TT], b_q(2))
        mulred(x_tm, TT, w_row(3), r_tm[:, TT:], "r_tt")
        nc.vector.tensor_scalar_add(r_tm[:, TT:], r_tm[:, TT:], b_q(3))

        # ---- main loop: strided 256-row blocks -----------------------------
        # spatial block t (t=0..7): rows {16p+t} (h=0) and {16p+t+8} (h=1)
        # temporal block t (t=0..1): rows 2048 + {4p+t} and 2048 + {4p+t+2}
        def block(k, t, base, nt, col, rst, hs, split=False):
            tmp = tmpp.tile([128, 2 * W], F16, name=f"tmp{k}", tag="tmp")
            ot = outp.tile([128, 2 * W], F16, name=f"ot{k}", tag="ot")
            quad = adj[base : base + 128 * nt, :]
            for h in range(2):
                o = h * W
                s = t + h * hs
                rn = rst[:, s : s + 1]
                rt = rst[:, nt + s : nt + s + 1]
                nc.vector.tensor_scalar(
                    tmp[:, o + N : o + W], col[:, N:W], rt, 0.0, OP.add, OP.max
                )
                nc.vector.tensor_scalar(
                    tmp[:, o : o + N], col[:, 0:N], rn, 0.0, OP.add, OP.max
                )
                if split:
                    nc.scalar.activation(ot[:, o : o + W], tmp[:, o : o + W], AF.Tanh)
                    nc.sync.dma_start(
                        quad.rearrange("(p r) w -> p r w", p=128)[:, s : s + 1, :],
                        ot[:, o : o + W].rearrange("p (r w) -> p r w", r=1),
                    )
            if not split:
                nc.scalar.activation(ot[:], tmp[:], AF.Tanh)
                # partition p -> rows base + nt*p + t and base + nt*p + t + hs
                nc.sync.dma_start(
                    quad.rearrange("(p g r) w -> p g r w", p=128, g=2)[
                        :, :, t : t + 1, :
                    ],
                    ot[:].rearrange("p (g w) -> p g w", g=2).unsqueeze(2),
                )

        for t in range(NT // 2):
            block(t, t, 0, NT, col_sp, r_sp, NT // 2, split=(t == 0))
        for t in range(TT // 2):
            block(8 + t, t, N, TT, col_tm, r_tm, TT // 2,
                  split=(t == TT // 2 - 1))


def build_nc(num_devices=8):
    nc = bacc.Bacc(
        "TRN2",
        target_bir_lowering=False,
        debug=False,
        enable_asserts=True,
        num_devices=num_devices,
    )
    sp = nc.dram_tensor("spatial_nodes", (N, D), F32, kind="ExternalInput").ap()
    tm = nc.dram_tensor("temporal_nodes", (T, D), F32, kind="ExternalInput").ap()
    wp = nc.dram_tensor("wpack", (260,), F32, kind="ExternalInput").ap()
    scr = {
        "sp": nc.dram_tensor("scr_sp", (W,), F16, kind="Internal").ap(),
        "tm": nc.dram_tensor("scr_tm", (W,), F16, kind="Internal").ap(),
    }
    adj = nc.dram_tensor("adj", (W, W), F16, kind="ExternalOutput").ap()

    with tile.TileContext(nc) as tc:
        _emit(tc, sp, tm, wp, scr, adj)
    nc.compile()
    return nc


def make_in_maps(inputs):
    wpack = np.concatenate(
        [np.asarray(inputs[f"w_{nm}"], np.float32).reshape(-1) for nm in QUADS]
        + [np.asarray(inputs[f"b_{nm}"], np.float32).reshape(-1) for nm in QUADS]
    )
    in_maps = []
    for b in range(B):
        m = {
            "spatial_nodes": np.ascontiguousarray(inputs["spatial_nodes"][b], np.float32),
            "temporal_nodes": np.ascontiguousarray(inputs["temporal_nodes"][b], np.float32),
            "wpack": wpack,
        }
        in_maps.append(m)
    return in_maps


_NC = {}


def run(inputs, trace=False, trace_cores=None):
    if 8 not in _NC:
        _NC[8] = build_nc(8)
    res = run_bass_kernel_spmd(
        _NC[8], make_in_maps(inputs), core_ids=list(range(B)), trace=trace,
        trace_cores=trace_cores,
    )
    out = np.stack(
        [res.results[i]["adj"].astype(np.float32) for i in range(B)], axis=0
    )
    return out, res


def kernel(**inputs) -> np.ndarray:
    out, _ = run(inputs, trace=False)
    return out
